# revision 25
# baseline (speedup 1.0000x reference)
"""Multi-head self-attention (B=2, S=2048, D=1024, H=16) on 8 trn2 cores.

Sharding: core c = b*4 + g  (b = batch, g = head-group of 4 heads).
Each core computes, for its batch b and local heads l=0..3:
  Qt = (Wq_g^T x_b^T + bq_g),  Kt likewise   -> [256, 2048] feature-major
  V  = x_b Wv_g                               -> per 128-key block [128, 4, 65]
                                                 (ones column for rowsum)
  scoresT[k,q] = sum_d Kt[d,k] Qt[d,q]        per 128-key block, single head
  expT = exp(0.125 * scoresT)                 (ACT, which does nothing else)
  ctxT/rowsum via PV matmul with V||ones      -> psum [65, 512] x2, full
                                                 128-key contraction
  ctxT normalized by 1/rowsum (DMA partition-broadcast + DVE mult)
  y = ctx_g @ Wo_g                            -> [2048, 1024]

Scheduling: the TRN2 PE clock throttles unless the tensor engine stays
continuously busy (full 2.4GHz only after 3us without a gap).  The exp
stream paces stage B at ~1.1us per 128x1024 block while the lean attention
matmuls only need ~0.85us, so deferred useful work fills each block's gap.
Only work needed before the first exp runs in stage A (K-t0, V, Q-qh0);
everything else streams through stage B as per-block fill:
  qh0: K-t1 projection, then Q-qh1 projection      (1 op/block)
  qh1: output projection as t=0 half -> f16 partial, t=1 half + DVE add
       (2 ops/block), chained so each piece lands after its deps resolve
  tail: t=1 output projection for q-half 1
Host: Y[b] = sum_g y_partial + (bo + bv @ Wo).
"""

import itertools
import sys

sys.path.insert(0, "/opt/trn_rl_repo")

import numpy as np

import concourse.bass as bass
import concourse.mybir as mybir
import concourse.tile as tile

F32 = mybir.dt.float32
F32R = mybir.dt.float32r
BF16 = mybir.dt.bfloat16
F16 = mybir.dt.float16
MMDT = F16                     # dtype for all matmul operands (f16: 1 cyc/row like bf16, 10-bit mantissa)
AF = mybir.ActivationFunctionType

D = 1024          # d_model
S = 2048          # sequence length
HPC = 4           # heads per core
DK = 64           # head dim
E = HPC * DK      # 256 features per core
N_CORES = 8


_ENGINE_OPS = {
    "InstMatmult", "InstActivation", "InstTensorCopy", "InstTensorTensor",
    "InstReciprocal", "InstTensorReduce", "InstMemset", "InstIota",
    "InstTensorScalarPtr", "InstTranspose", "InstLdweights",
    "InstDMACopy", "InstDrain", "InstNoOp",
}


def _legalize_matmul_waits(nc):
    """walrus allows at most 1 sync wait on engine compute instructions; Tile
    sometimes emits more. Move the excess onto EventSemaphore instructions
    (cap 2 each) placed immediately before in same-engine program order."""
    for f in nc.m.functions:
        for bb in f.blocks:
            out = []
            changed = False
            for i in bb.instructions:
                si = getattr(i, "sync_info", None)
                if (
                    type(i).__name__ in _ENGINE_OPS
                    and si is not None
                    and si.on_wait
                    and len(si.on_wait) > 1
                ):
                    waits = list(si.on_wait)
                    excess, keep = waits[:-1], waits[-1:]
                    for c in range(0, len(excess), 2):
                        ev = mybir.InstEventSemaphore(
                            name=f"{i.name}-mmw{c}", ins=[], outs=[]
                        )
                        ev.engine = i.engine
                        ev.sync_info = mybir.SyncInfo(
                            on_wait=excess[c:c + 2], on_update=[]
                        )
                        out.append(ev)
                    i.sync_info = mybir.SyncInfo(
                        on_wait=keep, on_update=list(si.on_update)
                    )
                    changed = True
                out.append(i)
            if changed:
                bb.instructions = out


def build_nc(legalize=True):
    nc = bass.Bass()

    xt = nc.dram_tensor("xt", [D, S], MMDT, kind="ExternalInput")
    wq = nc.dram_tensor("wq", [D, E], MMDT, kind="ExternalInput")
    wk = nc.dram_tensor("wk", [D, E], MMDT, kind="ExternalInput")
    wv = nc.dram_tensor("wv", [D, E], MMDT, kind="ExternalInput")
    wo = nc.dram_tensor("wo", [E, D], MMDT, kind="ExternalInput")
    bq = nc.dram_tensor("bq", [E], F32, kind="ExternalInput")
    bk = nc.dram_tensor("bk", [E], F32, kind="ExternalInput")
    y = nc.dram_tensor("y", [S, D], F32, kind="ExternalOutput")

    KT = D // 128     # 8 k-tiles over d_model
    QC = S // 512     # 4 q-chunks of 512
    SC = S // 128     # 16 seq chunks of 128
    ET = E // 128     # 2 feature tiles
    CB = S // 128     # 16 key blocks of 128

    with tile.TileContext(nc) as tc:
        with tc.tile_pool(name="persist", bufs=1) as pp:
            # ---- persistent tiles ----
            qt_sb = [pp.tile([128, S], MMDT, tag=f"qt{t}", name=f"qt{t}") for t in range(ET)]
            kt_sb = [pp.tile([128, S], MMDT, tag=f"kt{t}", name=f"kt{t}") for t in range(ET)]
            # v_sb[c]: V rows for 128-key block c, per local head l, with a
            # ones column at the end so PV also produces the softmax rowsum.
            v_sb = [pp.tile([128, HPC, DK + 1], MMDT, tag=f"v{c}", name=f"v{c}")
                    for c in range(CB)]
            ctx_sb = [pp.tile([128, S], MMDT, tag=f"ctx{t}", name=f"ctx{t}") for t in range(ET)]
            wo_sb = [pp.tile([128, D], MMDT, tag=f"wo{t}", name=f"wo{t}") for t in range(ET)]
            bq_sb = pp.tile([128, ET], F32, tag="bq")
            bk_sb = pp.tile([128, ET], F32, tag="bk")
            # x / W tiles stay resident: parts of the projections run inside
            # stage B as PE fill.
            xt_sb = [pp.tile([128, S], MMDT, tag=f"xt{k}", name=f"xt{k}") for k in range(KT)]
            wq_sb = pp.tile([128, KT, E], MMDT, tag="wq")
            wk_sb = pp.tile([128, KT, E], MMDT, tag="wk")
            wv_sb = pp.tile([128, KT, E], MMDT, tag="wv")
            # t=0 partials of the output projection, held until the t=1 pass
            ys0_sb = [pp.tile([128, 1024], MMDT, tag=f"ys0_{i}", name=f"ys0_{i}")
                      for i in range(8)]

            # DMA order follows first use: K projection starts after the
            # first (wk, xt) pair lands.
            for k in range(KT):
                nc.sync.dma_start(wk_sb[:, k, :], wk[k * 128:(k + 1) * 128, :])
                nc.sync.dma_start(xt_sb[k], xt[k * 128:(k + 1) * 128, :])
            nc.sync.dma_start(bk_sb, bk.rearrange("(t p) -> p t", p=128))
            for k in range(KT):
                nc.sync.dma_start(wv_sb[:, k, :], wv[k * 128:(k + 1) * 128, :])
            for k in range(KT):
                nc.sync.dma_start(wq_sb[:, k, :], wq[k * 128:(k + 1) * 128, :])
            nc.sync.dma_start(bq_sb, bq.rearrange("(t p) -> p t", p=128))
            for t in range(ET):
                nc.sync.dma_start(wo_sb[t], wo[t * 128:(t + 1) * 128, :])

            ones_sb = pp.tile([128, HPC], F32, tag="ones")
            nc.vector.memset(ones_sb, 1.0)
            for c in range(CB):
                nc.vector.tensor_copy(v_sb[c][:, :, DK:DK + 1],
                                      ones_sb[:, :, None])

            # ---- stage A: K-t0, V, and the q-half-0 Q projection ----
            with (
                tc.tile_pool(name="psA", bufs=8, space="PSUM") as psA,
            ):
                # Qt/Kt: feature-major [e, s];  out = W_tile^T @ xt.
                # k-outer with resident psum accumulators so the first
                # matmuls start as soon as xt[0]/w[0] land. Bias-add eviction
                # on DVE (ACT is reserved for exp).
                def emit_proj(wi, ts, qcs):
                    w_sb, b_sb, dst = (
                        (wq_sb, bq_sb, qt_sb), (wk_sb, bk_sb, kt_sb))[wi]
                    pss = {
                        (t, qc): psA.tile([128, 512], F32, tag="proj",
                                          name=f"pp{wi}_{t}_{qc}")
                        for t in ts for qc in qcs
                    }
                    for k in range(KT):
                        for t in ts:
                            for qc in qcs:
                                nc.tensor.matmul(
                                    pss[t, qc],
                                    w_sb[:, k, t * 128:(t + 1) * 128],
                                    xt_sb[k][:, qc * 512:(qc + 1) * 512],
                                    start=(k == 0),
                                    stop=(k == KT - 1),
                                )
                    for t in ts:
                        for qc in qcs:
                            nc.vector.tensor_scalar_add(
                                dst[t][:, qc * 512:(qc + 1) * 512],
                                pss[t, qc],
                                b_sb[:, t:t + 1],
                            )

                # V: seq-major [s, e];  out = xt_tile^T @ wv. Each 128-seq
                # psum chunk is exactly one key block; evict via DVE copy.
                def emit_v(s_range):
                    for s in s_range:
                        ps = psA.tile([128, E], F32, tag="proj",
                                      name=f"vps{s}")
                        for k in range(KT):
                            nc.tensor.matmul(
                                ps,
                                xt_sb[k][:, s * 128:(s + 1) * 128],
                                wv_sb[:, k, :],
                                start=(k == 0),
                                stop=(k == KT - 1),
                            )
                        nc.vector.tensor_copy(
                            v_sb[s][:, :, 0:DK],
                            ps.rearrange("p (h d) -> p h d", d=DK))

                emit_proj(1, [0], range(QC))     # K, t=0 feature half
                emit_v(range(0, SC))             # V (full)
                emit_proj(0, [0], [0, 1])        # Q, t=0, q-half 0

            # ---- stage B: attention, with fill work interleaved ----
            with (
                tc.tile_pool(name="stageB", bufs=3) as pb,
                tc.tile_pool(name="dramB", bufs=3, space="DRAM") as dramB,
                tc.tile_pool(name="psS", bufs=2, space="PSUM") as psS,
                tc.tile_pool(name="psC", bufs=2, space="PSUM") as psC,
                tc.tile_pool(name="psFY", bufs=2, space="PSUM") as psFY,
            ):
                # Scores for a single head: stationary Kt [64, 128] (64-wide
                # contraction costs the same as 128 — cost is moving rows),
                # psum [128 keys, 1024 q]. PV then contracts the full 128
                # keys against V||ones in one pass per 512-q chunk.
                def emit_scores(t, hp, qh, c):
                    sc_ps = psS.tile([128, 1024], F32, tag="sc",
                                     name=f"sc{t}_{hp}_{qh}_{c}")
                    p0 = hp * 64
                    for j in range(2):
                        nc.tensor.matmul(
                            sc_ps[:, j * 512:(j + 1) * 512],
                            kt_sb[t][p0:p0 + 64, c * 128:(c + 1) * 128],
                            qt_sb[t][p0:p0 + 64,
                                     qh * 1024 + j * 512:
                                     qh * 1024 + (j + 1) * 512],
                            start=True, stop=True,
                        )
                    return sc_ps

                # PE fill generators: one yield = one engine micro-op, so a
                # single fill op can be slotted into each attention block.
                def proj_fill_gen(wi, ts, qcs):
                    w_sb, b_sb, dst = (
                        (wq_sb, bq_sb, qt_sb), (wk_sb, bk_sb, kt_sb))[wi]
                    for t in ts:
                        for qc in qcs:
                            ps = psFY.tile([128, 512], F32, tag="fy",
                                           name=f"pf{wi}_{t}_{qc}")
                            for k in range(KT):
                                nc.tensor.matmul(
                                    ps,
                                    w_sb[:, k, t * 128:(t + 1) * 128],
                                    xt_sb[k][:, qc * 512:(qc + 1) * 512],
                                    start=(k == 0),
                                    stop=(k == KT - 1),
                                )
                                yield
                            nc.vector.tensor_scalar_add(
                                dst[t][:, qc * 512:(qc + 1) * 512],
                                ps,
                                b_sb[:, t:t + 1],
                            )
                            yield

                def y_t0_gen(qts):
                    # Output projection, t=0 feature half: psum -> f16 partial
                    for qt in qts:
                        ys0 = ys0_sb[qt % 8]
                        for n in range(2):
                            yp = psFY.tile([128, 512], F32, tag="fy",
                                           name=f"yp0_{qt}_{n}")
                            nc.tensor.matmul(
                                yp,
                                ctx_sb[0][:, qt * 128:(qt + 1) * 128],
                                wo_sb[0][:, n * 512:(n + 1) * 512],
                                start=True, stop=True,
                            )
                            yield
                            nc.vector.tensor_copy(
                                ys0[:, n * 512:(n + 1) * 512], yp)
                            yield

                def y_t1_gen(qts):
                    # Output projection, t=1 half; DVE adds the t=0 partial
                    # during eviction, then one full-row DMA per chunk.
                    for qt in qts:
                        ys0 = ys0_sb[qt % 8]
                        ys = pb.tile([128, 1024], F32, tag="ysf",
                                     name=f"ysf{qt}", bufs=3)
                        for n in range(2):
                            yp = psFY.tile([128, 512], F32, tag="fy",
                                           name=f"yp1_{qt}_{n}")
                            nc.tensor.matmul(
                                yp,
                                ctx_sb[1][:, qt * 128:(qt + 1) * 128],
                                wo_sb[1][:, n * 512:(n + 1) * 512],
                                start=True, stop=True,
                            )
                            yield
                            nc.vector.tensor_add(
                                ys[:, n * 512:(n + 1) * 512],
                                yp,
                                ys0[:, n * 512:(n + 1) * 512],
                            )
                            yield
                        nc.sync.dma_start(y[qt * 128:(qt + 1) * 128, :], ys)
                        yield

                # qh0 fill: K-t1 (needed by heads 2,3 from block 32), then
                # the q-half-1 Q projection.  qh1 fill: output projection for
                # qh0 (t=0 then t=1+add), then the t=0 half for qh1.
                chain_a = itertools.chain(
                    proj_fill_gen(0, [1], [0, 1]),     # Q t=1 qh0 (block 32)
                    proj_fill_gen(1, [1], range(QC)),  # K t=1 (block 32+)
                    proj_fill_gen(0, [0, 1], [2, 3]),  # Q qh1 (by qh1)
                )
                chain_b = itertools.chain(
                    y_t0_gen(range(0, 8)),
                    y_t1_gen(range(0, 8)),
                    y_t0_gen(range(8, 16)),
                )
                stages = {0: (chain_a, 2), 1: (chain_b, 2)}

                for qh in range(2):               # q halves of 1024
                    fill, pulls = stages[qh]
                    for l in range(HPC):          # local head
                        t, hp = l // 2, l % 2
                        ctx_ps = [
                            psC.tile([DK + 1, 512], F32, tag="ctx",
                                     name=f"ctxps{l}_{qh}_{j}")
                            for j in range(2)
                        ]
                        sc_ps = emit_scores(t, hp, qh, 0)
                        for c in range(CB):
                            # software pipeline: next block's scores go to
                            # the PE queue before this block's PV so PE isn't
                            # head-of-line blocked on the exp result; the
                            # fill ops land in the exp-wait gap.
                            sc_next = (emit_scores(t, hp, qh, c + 1)
                                       if c + 1 < CB else None)
                            for _ in range(pulls):
                                next(fill, None)
                            ex = pb.tile([128, 1024], MMDT, tag="ex",
                                         name=f"ex{l}_{qh}_{c}", bufs=6)
                            nc.scalar.activation(ex, sc_ps, AF.Exp,
                                                 scale=0.125)
                            for j in range(2):
                                nc.tensor.matmul(
                                    ctx_ps[j],
                                    v_sb[c][:, l, :],
                                    ex[:, j * 512:(j + 1) * 512],
                                    start=(c == 0),
                                    stop=(c == CB - 1),
                                )
                            sc_ps = sc_next
                        # Evict psum to SBUF right away (frees the ctx banks),
                        # then normalize from staging off the critical path.
                        stg = pb.tile([DK + 1, 1024], F32, tag="stg",
                                      name=f"stg{l}_{qh}")
                        for j in range(2):
                            nc.vector.tensor_copy(
                                stg[:, j * 512:(j + 1) * 512],
                                ctx_ps[j],
                            )
                        # reciprocal of the rowsum on 64 partitions
                        # ([64, 16] via DRAM scatter) — ~40x faster than
                        # on the natural [1, 1024] single-partition row.
                        rs_dr = dramB.tile(
                            [1, 1024], F32, tag="rs_dr",
                            name=f"rsdr{l}_{qh}",
                        )
                        nc.sync.dma_start(rs_dr, stg[DK:DK + 1, :])
                        rs64 = pb.tile([64, 16], F32, tag="rs64",
                                       name=f"rs64{l}_{qh}")
                        nc.sync.dma_start(
                            rs64, rs_dr.rearrange("o (p f) -> (o p) f", f=16)
                        )
                        rc64 = pb.tile([64, 16], F32, tag="rc64",
                                       name=f"rc64{l}_{qh}")
                        nc.vector.reciprocal(rc64, rs64)
                        rc_dr = dramB.tile(
                            [1, 1024], F32, tag="rc_dr",
                            name=f"rcdr{l}_{qh}",
                        )
                        nc.sync.dma_start(
                            rc_dr.rearrange("o (p f) -> (o p) f", f=16), rc64
                        )
                        rb = pb.tile([64, 1024], F32, tag="rb",
                                     name=f"rb{l}_{qh}")
                        nc.sync.dma_start(
                            rb, rc_dr.to_broadcast([64, 1024])
                        )
                        nc.vector.tensor_mul(
                            ctx_sb[t][hp * 64:hp * 64 + 64,
                                      qh * 1024:(qh + 1) * 1024],
                            stg[0:DK, :],
                            rb,
                        )
                    if qh == 0:
                        # drain leftover projection fill before q-half 1
                        # needs it; the burst builds PE backlog, keeping the
                        # clock warm across the boundary.
                        for _ in fill:
                            pass
            # ---- tail: t=1 output projection for q-half 1 ----
            # Own scope: stage-B psum pools are closed, so the tail gets 4
            # psum buffers and the matmul/add/DMA chains pipeline deeply.
            # ACT is idle after the last exp, so half the psum evictions run
            # there (identity-matmul folds the t=0 partial into the psum
            # first, making the eviction a pure copy).
            with (
                tc.tile_pool(name="tailP", bufs=3) as pt,
                tc.tile_pool(name="psT", bufs=4, space="PSUM") as psT,
            ):
                for qt in range(8, 16):
                    ys0 = ys0_sb[qt % 8]
                    ys = pt.tile([128, 1024], F32, tag="yst",
                                 name=f"yst{qt}")
                    for n in range(2):
                        yp = psT.tile([128, 512], F32, tag="yt",
                                      name=f"ypt{qt}_{n}")
                        nc.tensor.matmul(
                            yp,
                            ctx_sb[1][:, qt * 128:(qt + 1) * 128],
                            wo_sb[1][:, n * 512:(n + 1) * 512],
                            start=True, stop=True,
                        )
                        nc.vector.tensor_add(
                            ys[:, n * 512:(n + 1) * 512],
                            yp,
                            ys0[:, n * 512:(n + 1) * 512],
                        )
                    nc.sync.dma_start(y[qt * 128:(qt + 1) * 128, :], ys)
    if legalize:
        _legalize_matmul_waits(nc)
    return nc


_NC_CACHE = None


def _get_nc():
    global _NC_CACHE
    if _NC_CACHE is None:
        _NC_CACHE = build_nc()
    return _NC_CACHE


def make_in_maps(inputs):
    mmnp = mybir.dt.np(MMDT)
    x = np.asarray(inputs["x"], dtype=np.float32)
    Wq = np.asarray(inputs["Wq"], dtype=np.float32)
    Wk = np.asarray(inputs["Wk"], dtype=np.float32)
    Wv = np.asarray(inputs["Wv"], dtype=np.float32)
    Wo = np.asarray(inputs["Wo"], dtype=np.float32)
    bq = np.asarray(inputs["bq"], dtype=np.float32)
    bk = np.asarray(inputs["bk"], dtype=np.float32)

    in_maps = []
    for c in range(N_CORES):
        b, g = c // 4, c % 4
        sl = slice(g * E, (g + 1) * E)
        in_maps.append({
            "xt": np.ascontiguousarray(x[b].T).astype(mmnp),
            "wq": np.ascontiguousarray(Wq[:, sl]).astype(mmnp),
            "wk": np.ascontiguousarray(Wk[:, sl]).astype(mmnp),
            "wv": np.ascontiguousarray(Wv[:, sl]).astype(mmnp),
            "wo": np.ascontiguousarray(Wo[sl, :]).astype(mmnp),
            "bq": np.ascontiguousarray(bq[sl]),
            "bk": np.ascontiguousarray(bk[sl]),
        })
    return in_maps


def kernel(x, Wq, bq, Wk, bk, Wv, bv, Wo, bo):
    from concourse.bass_utils import run_bass_kernel_spmd

    x = np.asarray(x, dtype=np.float32)
    Wv = np.asarray(Wv, dtype=np.float32)
    Wo = np.asarray(Wo, dtype=np.float32)
    bv = np.asarray(bv, dtype=np.float32)
    bo = np.asarray(bo, dtype=np.float32)

    B = x.shape[0]
    nc = _get_nc()
    in_maps = make_in_maps({
        "x": x, "Wq": Wq, "Wk": Wk, "Wv": Wv, "Wo": Wo, "bq": bq, "bk": bk,
    })

    res = run_bass_kernel_spmd(nc, in_maps, core_ids=list(range(N_CORES)))

    bias_total = bo + bv @ Wo  # [D]
    out = np.zeros((B, S, D), dtype=np.float32)
    for c in range(N_CORES):
        out[c // 4] += res.results[c]["y"]
    out += bias_total[None, None, :]
    return out
